# revision 8
# baseline (speedup 1.0000x reference)
"""MultiHeadDoubleAttention on TRN2 — fp8 DoubleRow conv + bf16 attention.

Data-parallel over batch: 8 cores x 16 batch each.

Conv: 15x15 masked conv on 8x8 grid = 65 shift taps of channel matmuls.
All conv matmuls run fp8-e4m3 with perf_mode=DoubleRow: the 256-channel
contraction is packed 2-per-PE-cell ([c, 2, .] operands), so one matmul
per (tap, o-half, psum-bank piece) does the full input-channel reduction
at ~N/2.4ns. Weights are scaled x256 into e4m3 range; copy-out rescales
by 1/256 (end-to-end rel err ~6e-3, tolerance 2e-2 — the output norm is
dominated by the bo bias, which dilutes conv-path error ~5x).

Attention is bf16: QK with 4x2 tile-position packing, one batched exp
per (oh,hp) psum bank, paired 128x128 PE transposes for vh and attn-out,
AV output packed to partition offset 64*par so all normalize/copy ops
stay lane-local.
"""
import sys
sys.path.insert(0, '/opt/trn_rl_repo')
import numpy as np
import ml_dtypes

import concourse.bass as bass
import concourse.bacc as bacc
import concourse.mybir as mybir
import concourse.bass_utils as bass_utils
from concourse.tile import TileContext

F32 = mybir.dt.float32
F32R = mybir.dt.float32r
BF16 = mybir.dt.bfloat16
FP8 = mybir.dt.float8e4
DR = mybir.MatmulPerfMode.DoubleRow
AF = mybir.ActivationFunctionType

B, D, H, DK = 128, 256, 8, 32
NCORES = 8
BL = B // NCORES          # 16 batch per core
NPIX = 64
WS = 256.0                # fp8 weight scale
RS = 1.0 / np.sqrt(DK)


def hollow_mask():
    m = np.ones((15, 15), np.float32)
    for c in range(5):
        m[1 + c:7, c] = 0; m[8:14 - c, c] = 0
        m[c, 1 + c:7] = 0; m[c, 8:14 - c] = 0
        m[1 + c:7, 14 - c] = 0; m[8:14 - c, 14 - c] = 0
        m[14 - c, 1 + c:7] = 0; m[14 - c, 8:14 - c] = 0
    return m


def tap_schedule():
    """All 65 unmasked taps as (sr, sc, ar0, hr, ac0, wc), biggest first."""
    m = hollow_mask()
    taps = []
    for di in range(15):
        for dj in range(15):
            if not m[di, dj]:
                continue
            sr, sc = di - 7, dj - 7
            ar0, ar1 = max(0, sr), min(7, 7 + sr)
            ac0, ac1 = max(0, sc), min(7, 7 + sc)
            taps.append((sr, sc, ar0, ar1 - ar0 + 1, ac0, ac1 - ac0 + 1))
    taps.sort(key=lambda e: -(e[3] * e[5]))
    return taps


TAPS = tap_schedule()
NTAP = len(TAPS)


def tap_pieces(sr, sc, ar0, hr, ac0, wc):
    """Split a tap's output rect at the ir=4 psum-bank boundary.
    Returns list of (bank, irb, ar0, ph, ic0, ac0, wc)."""
    ir0 = ar0 - sr
    ic0 = ac0 - sc
    pieces = []
    lo, hi = ir0, ir0 + hr
    if lo < 4:
        ph = min(hi, 4) - lo
        pieces.append((0, lo, lo + sr, ph, ic0, ac0, wc))
    if hi > 4:
        p0 = max(lo, 4)
        ph = hi - p0
        pieces.append((1, p0 - 4, p0 + sr, ph, ic0, ac0, wc))
    return pieces


PIECES = [tap_pieces(*t) for t in TAPS]
BANK_TOTALS = {0: 0, 1: 0}
for pl in PIECES:
    for p in pl:
        BANK_TOTALS[p[0]] += 1


def build_kernel():
    nc = bacc.Bacc("TRN2", target_bir_lowering=False, debug=False,
                   num_devices=NCORES)
    dt = {}
    for nm in ("q", "k", "v"):
        dt[f"x{nm}"] = nc.dram_tensor(f"x{nm}", [128, 2, 8, 8, BL], FP8,
                                      kind="ExternalInput")
        dt[f"w{nm}"] = nc.dram_tensor(f"w{nm}", [128, NTAP, 2, 256], FP8,
                                      kind="ExternalInput")
        dt[f"bias{nm}"] = nc.dram_tensor(f"bias{nm}", [128, 2], F32,
                                         kind="ExternalInput")
    dt["wo_t"] = nc.dram_tensor("wo_t", [128, 2, 256], BF16, kind="ExternalInput")
    dt["bo"] = nc.dram_tensor("bo", [1, 256], F32R, kind="ExternalInput")
    dt["ones"] = nc.dram_tensor("ones", [1, 128], F32R, kind="ExternalInput")
    dt["ident"] = nc.dram_tensor("ident", [128, 128], BF16, kind="ExternalInput")
    dt["out"] = nc.dram_tensor("out", [8, 128, 256], F32, kind="ExternalOutput")

    with TileContext(nc) as tc:
      with tc.tile_pool(name="persist", bufs=1) as pp:
        # ---- input / weight tiles ----
        x8 = {}
        w8 = {}
        bias_t = {}
        for nm in ("q", "k", "v"):
            x8[nm] = pp.tile([128, 2, 8, 8, BL], FP8, name=f"x{nm}")
            w8[nm] = pp.tile([128, NTAP, 2, 256], FP8, name=f"w{nm}")
            bias_t[nm] = pp.tile([128, 2], F32, name=f"bias{nm}_t")
        x1 = {nm: pp.tile([128, 2, 8, 8, BL], FP8, name=f"x1{nm}")
              for nm in ("q", "k", "v")}
        # conv2 outputs, bf16 [o-half 128, b, pix]
        hh = {nm: [pp.tile([128, BL, NPIX], BF16, name=f"h{nm}{h}")
                   for h in range(2)] for nm in ("q", "k", "v")}
        wo_tt = pp.tile([128, 2, 256], BF16, name="wo_tt")
        bo_t = pp.tile([1, 256], F32R, name="bo_t")
        ones_t = pp.tile([1, 128], F32R, name="ones_t")
        ident_t = pp.tile([128, 128], BF16, name="ident_t")

        # ---- DMAs (x first, then weights in chunks; wq first for P1) ----
        # tiny persist tensors first — P1's copy-out needs bias_q, so these
        # must not queue behind the 13.5MB weight stream
        for nm in ("q", "k", "v"):
            nc.sync.dma_start(bias_t[nm][:], dt[f"bias{nm}"].ap())
        nc.sync.dma_start(wo_tt[:], dt["wo_t"].ap())
        nc.sync.dma_start(bo_t[:], dt["bo"].ap())
        nc.sync.dma_start(ones_t[:], dt["ones"].ap())
        nc.sync.dma_start(ident_t[:], dt["ident"].ap())
        nc.sync.dma_start(x8["q"][:], dt["xq"].ap())
        def load_w(nm):
            t0 = 0
            while t0 < NTAP:
                n = min(2 if t0 == 0 else 8, NTAP - t0)
                nc.sync.dma_start(
                    w8[nm][:, t0:t0 + n],
                    dt[f"w{nm}"].ap()[:, t0:t0 + n])
                t0 += n
        load_w("q")
        nc.sync.dma_start(x8["k"][:], dt["xk"].ap())
        load_w("k")
        nc.sync.dma_start(x8["v"][:], dt["xv"].ap())
        load_w("v")

        # ---- conv pass: fp8 DoubleRow, per-oh waves ----
        def conv_pass(psp, wt, inputs, outs, tag, bufs=4, wave_cb=None):
            """wt: weight tile; inputs: list of x8-like tiles;
            outs: list of (kind, dest, bias) per input:
              kind 'relu' -> dest x1 tile (fp8), kind 'final' -> dest hh pair
            """
            for oh in range(2):
                ps = [[psp.tile([128, 4, 8, BL], F32, tag="cv",
                                name=f"{tag}ps{ii}{oh}{bk}", bufs=bufs)
                       for bk in range(2)] for ii in range(len(inputs))]
                done = {}
                for ti in range(NTAP):
                    lhsT = wt[:, ti, :, oh * 128:(oh + 1) * 128]
                    for ii, xt in enumerate(inputs):
                        for (bk, irb, ar0, ph, ic0, ac0, wc) in PIECES[ti]:
                            cnt = done.get((ii, bk), 0)
                            done[(ii, bk)] = cnt + 1
                            rhs = xt[:, :, ar0:ar0 + ph, ac0:ac0 + wc, :]
                            out = ps[ii][bk][:, irb:irb + ph, ic0:ic0 + wc, :]
                            nc.tensor.matmul(
                                out, lhsT, rhs,
                                start=(cnt == 0),
                                stop=(cnt == BANK_TOTALS[bk] - 1),
                                perf_mode=DR)
                for ii, (kind, dest, bias) in enumerate(outs):
                    for bk in range(2):
                        if kind == "relu":
                            nc.scalar.activation(
                                dest[:, oh, bk * 4:(bk + 1) * 4, :, :],
                                ps[ii][bk][:], AF.Relu,
                                bias=bias[:, oh:oh + 1], scale=1.0 / WS)
                        else:
                            nc.scalar.activation(
                                dest[oh][:, :, bk * 32:(bk + 1) * 32],
                                ps[ii][bk][:].rearrange("c pr pc b -> c b (pr pc)"),
                                AF.Identity,
                                bias=bias[:, oh:oh + 1], scale=1.0 / WS)
                if wave_cb is not None:
                    wave_cb(oh)

        kh, qh, vh = hh["k"], hh["q"], hh["v"]
        # E_t: [128=(par,64k), oh, hp, b2, 64q] bf16
        E_t = pp.tile([128, 2, 4, BL // 2, NPIX], BF16, name="E_t")
        # VT: [128=(par,64k), b2, h, 33] bf16, col 32 = ones
        VT = pp.tile([128, BL // 2, H, 33], BF16, name="VT")
        nc.vector.memset(VT[:, :, :, 32:33], 1.0)
        # OA: [128=(par,64q), b2, oh, 128c] bf16 (normalized attn out)
        OA = pp.tile([128, BL // 2, 2, 128], BF16, name="OA")
        rcp = pp.tile([128, BL // 2, H], F32, name="rcp")
        concat = [pp.tile([128, BL, NPIX], BF16, name=f"concat{h}")
                  for h in range(2)]
        out_sb = pp.tile([128, 8, 256], F32, name="out_sb")

        with tc.tile_pool(name="pscv", bufs=1, space="PSUM") as cvp:
            # P1: q conv1;  P2: k conv1;  P3: k/q conv2 (wk);
            conv_pass(cvp, w8["q"], [x8["q"]], [("relu", x1["q"], bias_t["q"])], "p1")
            conv_pass(cvp, w8["k"], [x8["k"]], [("relu", x1["k"], bias_t["k"])], "p2")
            conv_pass(cvp, w8["k"], [x1["k"], x1["q"]],
                      [("final", hh["k"], bias_t["k"]),
                       ("final", hh["q"], bias_t["k"])], "p3")

        with tc.tile_pool(name="psqk", bufs=1, space="PSUM") as qkp:
            # ---- QK + exp (overlaps P4 convs) ----
            for oh in range(2):
                for hp in range(4):
                    pst = qkp.tile([128, BL // 2, 64], F32, tag="pst",
                                   name=f"pst{oh}{hp}", bufs=2)
                    for b2 in range(BL // 2):
                        for par in range(2):
                            b = 2 * b2 + par
                            nc.tensor.matmul(
                                pst[64 * par:64 * par + 64, b2, :],
                                kh[oh][hp * 32:(hp + 1) * 32, b, :],
                                qh[oh][hp * 32:(hp + 1) * 32, b, :],
                                start=True, stop=True,
                                tile_position=(32 * hp, 64 * par))
                    nc.scalar.activation(E_t[:, oh, hp, :, :], pst[:],
                                         AF.Exp, scale=RS)

            with tc.tile_pool(name="pstail", bufs=1, space="PSUM") as tlp:
                # per-oh attention block, interleaved into P5's conv waves
                def attn_block(oh):
                    for b2 in range(BL // 2):
                        pvt = tlp.tile([128, 128], BF16, tag="ptr",
                                       name=f"pvt{b2}{oh}", bufs=2)
                        nc.tensor.transpose(
                            pvt[:], vh[oh][:, 2 * b2:2 * b2 + 2, :], ident_t[:])
                        nc.vector.tensor_copy(
                            VT[:, b2, oh * 4:(oh + 1) * 4, 0:32],
                            pvt[:].rearrange("k (h d) -> k h d", h=4))
                        pso = tlp.tile([128, 4, 33], F32, tag="pso",
                                       name=f"pso{b2}{oh}", bufs=2)
                        for hp in range(4):
                            for par in range(2):
                                nc.tensor.matmul(
                                    pso[64 * par:64 * par + 64, hp, :],
                                    E_t[64 * par:64 * par + 64, oh, hp, b2, :],
                                    VT[64 * par:64 * par + 64, b2, oh * 4 + hp, :],
                                    start=True, stop=True)
                        nc.vector.reciprocal(
                            rcp[:, b2, oh * 4:(oh + 1) * 4],
                            pso[:, :, 32:33].rearrange("q h one -> q (h one)"))
                        for hp in range(4):
                            h = oh * 4 + hp
                            dst = OA[:, b2, oh, hp * 32:(hp + 1) * 32]
                            src = pso[:, hp, 0:32]
                            if hp % 2 == 0:
                                nc.scalar.activation(dst, src, AF.Copy,
                                                     scale=rcp[:, b2, h:h + 1])
                            else:
                                nc.vector.tensor_scalar_mul(dst, src,
                                                            rcp[:, b2, h:h + 1])
                        pot = tlp.tile([128, 128], BF16, tag="ptr",
                                       name=f"pot{b2}{oh}", bufs=2)
                        nc.tensor.transpose(pot[:], OA[:, b2, oh, :], ident_t[:])
                        nc.vector.tensor_copy(
                            concat[oh][:, 2 * b2:2 * b2 + 2, :],
                            pot[:].rearrange("c (b q) -> c b q", b=2))

                # P4: v conv1;  P5: v conv2 (attn tail interleaved per wave)
                conv_pass(tlp, w8["v"], [x8["v"]],
                          [("relu", x1["v"], bias_t["v"])], "p4", bufs=2)
                conv_pass(tlp, w8["v"], [x1["v"]],
                          [("final", hh["v"], bias_t["v"])], "p5", bufs=2,
                          wave_cb=attn_block)

            with tc.tile_pool(name="psprj", bufs=1, space="PSUM") as prp:
                # output projection: per 128-col block of (b, pix)
                for blk in range(8):
                    pspr = prp.tile([128, 256], F32, tag="pspr",
                                    name=f"pspr{blk}", bufs=2)
                    for oh in range(2):
                        cs = concat[oh][:].rearrange("c b p -> c (b p)")
                        nc.tensor.matmul(
                            pspr[:], cs[:, blk * 128:(blk + 1) * 128],
                            wo_tt[:, oh, :], start=(oh == 0), stop=False)
                    nc.tensor.matmul(pspr[:], ones_t[:], bo_t[:],
                                     start=False, stop=True)
                    nc.vector.tensor_copy(out_sb[:, blk, :], pspr[:])
                    nc.sync.dma_start(dt["out"][blk], out_sb[:, blk, :])
    nc.compile()
    return nc


# ---------------------------------------------------------------------------
# Host-side prep
# ---------------------------------------------------------------------------
def _to_fp8(a):
    return np.clip(np.asarray(a, np.float32), -240.0, 240.0).astype(
        ml_dtypes.float8_e4m3)


def prep_static(wk, bk, wq, bq, wv, bv, wo, bo):
    st = {}
    for nm, w, b in (("q", wq, bq), ("k", wk, bk), ("v", wv, bv)):
        w = np.asarray(w, np.float32)
        # [128c_lo-part, tap, 2(c-half), 256o] fp8, scaled x256
        wt = np.empty((128, NTAP, 2, 256), ml_dtypes.float8_e4m3)
        for ti, (sr, sc, *_r) in enumerate(TAPS):
            wtap = w[:, :, sr + 7, sc + 7].T * WS     # [c, o]
            wt[:, ti] = _to_fp8(wtap.reshape(2, 128, 256).transpose(1, 0, 2))
        st[f"w{nm}"] = np.ascontiguousarray(wt)
        st[f"bias{nm}"] = np.ascontiguousarray(
            np.asarray(b, np.float32).reshape(2, 128).T)
    st["wo_t"] = np.ascontiguousarray(
        np.asarray(wo, np.float32).T.reshape(2, 128, 256).transpose(1, 0, 2)
        .astype(ml_dtypes.bfloat16))
    st["bo"] = np.asarray(bo, np.float32).reshape(1, 256)
    st["ones"] = np.ones((1, 128), np.float32)
    st["ident"] = np.eye(128, dtype=ml_dtypes.bfloat16)
    return st


def prep_core_x(x, core):
    """x: [B, 8, 8, D] -> [128, 2, 8, 8, BL] fp8 (c_lo, c_half, pr, pc, b)."""
    xs = np.asarray(x[core * BL:(core + 1) * BL], np.float32)
    xs = xs.transpose(3, 1, 2, 0).reshape(2, 128, 8, 8, BL).transpose(
        1, 0, 2, 3, 4)
    return np.ascontiguousarray(_to_fp8(xs))


def make_in_maps(q, k, v, st):
    in_maps = []
    for core in range(NCORES):
        m = dict(st)
        m["xq"] = prep_core_x(q, core)
        m["xk"] = prep_core_x(k, core)
        m["xv"] = prep_core_x(v, core)
        in_maps.append(m)
    return in_maps


def gather_out(results):
    outs = [r["out"].reshape(BL, 8, 8, D) for r in results]
    return np.concatenate(outs, axis=0)


_NC_CACHE = None


def _get_nc():
    global _NC_CACHE
    if _NC_CACHE is None:
        _NC_CACHE = build_kernel()
    return _NC_CACHE


def kernel(q, k, v, wk, bk, wq, bq, wv, bv, wo, bo):
    nc = _get_nc()
    st = prep_static(wk, bk, wq, bq, wv, bv, wo, bo)
    in_maps = make_in_maps(np.asarray(q), np.asarray(k), np.asarray(v), st)
    res = bass_utils.run_bass_kernel_spmd(
        nc, in_maps, core_ids=list(range(NCORES)))
    return gather_out(res.results)


# revision 10
# speedup vs baseline: 1.0923x; 1.0923x over previous
"""MultiHeadDoubleAttention on TRN2 — fp8 DoubleRow conv + bf16 attention.

Data-parallel over batch: 8 cores x 16 batch each.

Conv: 15x15 masked conv on 8x8 grid = 65 shift taps of channel matmuls.
All conv matmuls run fp8-e4m3 with perf_mode=DoubleRow: the 256-channel
contraction is packed 2-per-PE-cell ([c, 2, .] operands), so one matmul
per (tap, o-half, psum-bank piece) does the full input-channel reduction
at ~N/2.4ns. Weights are scaled x256 into e4m3 range; copy-out rescales
by 1/256 (end-to-end rel err ~6e-3, tolerance 2e-2 — the output norm is
dominated by the bo bias, which dilutes conv-path error ~5x).

Attention is bf16: QK with 4x2 tile-position packing, one batched exp
per (oh,hp) psum bank, paired 128x128 PE transposes for vh and attn-out,
AV output packed to partition offset 64*par so all normalize/copy ops
stay lane-local.
"""
import sys
sys.path.insert(0, '/opt/trn_rl_repo')
import numpy as np
import ml_dtypes

import concourse.bass as bass
import concourse.bacc as bacc
import concourse.mybir as mybir
import concourse.bass_utils as bass_utils
from concourse.tile import TileContext

F32 = mybir.dt.float32
F32R = mybir.dt.float32r
BF16 = mybir.dt.bfloat16
FP8 = mybir.dt.float8e4
DR = mybir.MatmulPerfMode.DoubleRow
AF = mybir.ActivationFunctionType

B, D, H, DK = 128, 256, 8, 32
NCORES = 8
BL = B // NCORES          # 16 batch per core
NPIX = 64
WS = 256.0                # fp8 weight scale
RS = 1.0 / np.sqrt(DK)


def hollow_mask():
    m = np.ones((15, 15), np.float32)
    for c in range(5):
        m[1 + c:7, c] = 0; m[8:14 - c, c] = 0
        m[c, 1 + c:7] = 0; m[c, 8:14 - c] = 0
        m[1 + c:7, 14 - c] = 0; m[8:14 - c, 14 - c] = 0
        m[14 - c, 1 + c:7] = 0; m[14 - c, 8:14 - c] = 0
    return m


def tap_schedule():
    """All 65 unmasked taps as (sr, sc, ar0, hr, ac0, wc), biggest first."""
    m = hollow_mask()
    taps = []
    for di in range(15):
        for dj in range(15):
            if not m[di, dj]:
                continue
            sr, sc = di - 7, dj - 7
            ar0, ar1 = max(0, sr), min(7, 7 + sr)
            ac0, ac1 = max(0, sc), min(7, 7 + sc)
            taps.append((sr, sc, ar0, ar1 - ar0 + 1, ac0, ac1 - ac0 + 1))
    taps.sort(key=lambda e: -(e[3] * e[5]))
    return taps


TAPS = tap_schedule()
NTAP = len(TAPS)


def tap_pieces(sr, sc, ar0, hr, ac0, wc):
    """Split a tap's output rect at the ir=4 psum-bank boundary.
    Returns list of (bank, irb, ar0, ph, ic0, ac0, wc)."""
    ir0 = ar0 - sr
    ic0 = ac0 - sc
    pieces = []
    lo, hi = ir0, ir0 + hr
    if lo < 4:
        ph = min(hi, 4) - lo
        pieces.append((0, lo, lo + sr, ph, ic0, ac0, wc))
    if hi > 4:
        p0 = max(lo, 4)
        ph = hi - p0
        pieces.append((1, p0 - 4, p0 + sr, ph, ic0, ac0, wc))
    return pieces


PIECES = [tap_pieces(*t) for t in TAPS]
BANK_TOTALS = {0: 0, 1: 0}
for pl in PIECES:
    for p in pl:
        BANK_TOTALS[p[0]] += 1


def build_kernel():
    nc = bacc.Bacc("TRN2", target_bir_lowering=False, debug=False,
                   num_devices=NCORES)
    dt = {}
    for nm in ("q", "k", "v"):
        dt[f"x{nm}"] = nc.dram_tensor(f"x{nm}", [128, 2, 8, 8, BL], FP8,
                                      kind="ExternalInput")
        dt[f"w{nm}"] = nc.dram_tensor(f"w{nm}", [128, NTAP, 2, 256], FP8,
                                      kind="ExternalInput")
        dt[f"bias{nm}"] = nc.dram_tensor(f"bias{nm}", [128, 2], F32,
                                         kind="ExternalInput")
    dt["wo_t"] = nc.dram_tensor("wo_t", [128, 2, 256], BF16, kind="ExternalInput")
    dt["bo"] = nc.dram_tensor("bo", [1, 256], F32R, kind="ExternalInput")
    dt["ones"] = nc.dram_tensor("ones", [1, 128], F32R, kind="ExternalInput")
    dt["ident"] = nc.dram_tensor("ident", [128, 128], BF16, kind="ExternalInput")
    dt["out"] = nc.dram_tensor("out", [8, 128, 256], F32, kind="ExternalOutput")

    with TileContext(nc) as tc:
      with tc.tile_pool(name="persist", bufs=1) as pp:
        # ---- input / weight tiles ----
        x8 = {}
        w8 = {}
        bias_t = {}
        for nm in ("q", "k", "v"):
            x8[nm] = pp.tile([128, 2, 8, 8, BL], FP8, name=f"x{nm}")
            w8[nm] = pp.tile([128, NTAP, 2, 256], FP8, name=f"w{nm}")
            bias_t[nm] = pp.tile([128, 2], F32, name=f"bias{nm}_t")
        x1 = {nm: pp.tile([128, 2, 8, 8, BL], FP8, name=f"x1{nm}")
              for nm in ("q", "k", "v")}
        # conv2 outputs, bf16 [o-half 128, b, pix]
        hh = {nm: [pp.tile([128, BL, NPIX], BF16, name=f"h{nm}{h}")
                   for h in range(2)] for nm in ("q", "k", "v")}
        wo_tt = pp.tile([128, 2, 256], BF16, name="wo_tt")
        bo_t = pp.tile([1, 256], F32R, name="bo_t")
        ones_t = pp.tile([1, 128], F32R, name="ones_t")
        ident_t = pp.tile([128, 128], BF16, name="ident_t")

        # ---- DMAs (x first, then weights in chunks; wq first for P1) ----
        # tiny persist tensors first — P1's copy-out needs bias_q, so these
        # must not queue behind the 13.5MB weight stream
        for nm in ("q", "k", "v"):
            nc.sync.dma_start(bias_t[nm][:], dt[f"bias{nm}"].ap())
        nc.sync.dma_start(wo_tt[:], dt["wo_t"].ap())
        nc.sync.dma_start(bo_t[:], dt["bo"].ap())
        nc.sync.dma_start(ones_t[:], dt["ones"].ap())
        nc.sync.dma_start(ident_t[:], dt["ident"].ap())
        nc.sync.dma_start(x8["q"][:], dt["xq"].ap())
        def load_w(nm):
            t0 = 0
            while t0 < NTAP:
                n = min(2 if t0 == 0 else 8, NTAP - t0)
                nc.sync.dma_start(
                    w8[nm][:, t0:t0 + n],
                    dt[f"w{nm}"].ap()[:, t0:t0 + n])
                t0 += n
        load_w("q")
        nc.sync.dma_start(x8["k"][:], dt["xk"].ap())
        load_w("k")
        nc.sync.dma_start(x8["v"][:], dt["xv"].ap())
        load_w("v")

        # ---- conv pass: fp8 DoubleRow, per-oh waves ----
        def conv_pass(psp, wt, inputs, outs, tag, bufs=4, wave_cb=None):
            """wt: weight tile; inputs: list of x8-like tiles;
            outs: list of (kind, dest, bias) per input:
              kind 'relu' -> dest x1 tile (fp8), kind 'final' -> dest hh pair
            """
            for oh in range(2):
                ps = [[psp.tile([128, 4, 8, BL], F32, tag="cv",
                                name=f"{tag}ps{ii}{oh}{bk}", bufs=bufs)
                       for bk in range(2)] for ii in range(len(inputs))]
                done = {}
                for ti in range(NTAP):
                    lhsT = wt[:, ti, :, oh * 128:(oh + 1) * 128]
                    for ii, xt in enumerate(inputs):
                        for (bk, irb, ar0, ph, ic0, ac0, wc) in PIECES[ti]:
                            cnt = done.get((ii, bk), 0)
                            done[(ii, bk)] = cnt + 1
                            rhs = xt[:, :, ar0:ar0 + ph, ac0:ac0 + wc, :]
                            out = ps[ii][bk][:, irb:irb + ph, ic0:ic0 + wc, :]
                            nc.tensor.matmul(
                                out, lhsT, rhs,
                                start=(cnt == 0),
                                stop=(cnt == BANK_TOTALS[bk] - 1),
                                perf_mode=DR)
                for ii, (kind, dest, bias) in enumerate(outs):
                    for bk in range(2):
                        if kind == "relu":
                            nc.scalar.activation(
                                dest[:, oh, bk * 4:(bk + 1) * 4, :, :],
                                ps[ii][bk][:], AF.Relu,
                                bias=bias[:, oh:oh + 1], scale=1.0 / WS)
                        else:
                            nc.scalar.activation(
                                dest[oh][:, :, bk * 32:(bk + 1) * 32],
                                ps[ii][bk][:].rearrange("c pr pc b -> c b (pr pc)"),
                                AF.Identity,
                                bias=bias[:, oh:oh + 1], scale=1.0 / WS)
                if wave_cb is not None:
                    wave_cb(oh)

        kh, qh, vh = hh["k"], hh["q"], hh["v"]
        # E_t: [128=(par,64k), oh, hp, b2, 64q] bf16
        E_t = pp.tile([128, 2, 4, BL // 2, NPIX], BF16, name="E_t")
        # VT: [128=(par,64k), b2, h, 33] bf16, col 32 = ones
        VT = pp.tile([128, BL // 2, H, 33], BF16, name="VT")
        nc.vector.memset(VT[:, :, :, 32:33], 1.0)
        # OA: [128=(par,64q), b2, oh, 128c] bf16 (normalized attn out)
        OA = pp.tile([128, BL // 2, 2, 128], BF16, name="OA")
        rcp = pp.tile([128, BL // 2, H], F32, name="rcp")
        concat = [pp.tile([128, BL, NPIX], BF16, name=f"concat{h}")
                  for h in range(2)]
        out_sb = pp.tile([128, 8, 256], F32, name="out_sb")

        with tc.tile_pool(name="pscv", bufs=1, space="PSUM") as cvp:
            # P1: q conv1;  P2: k conv1;  P3: k/q conv2 (wk);
            conv_pass(cvp, w8["q"], [x8["q"]], [("relu", x1["q"], bias_t["q"])], "p1")
            conv_pass(cvp, w8["k"], [x8["k"]], [("relu", x1["k"], bias_t["k"])], "p2")
            conv_pass(cvp, w8["k"], [x1["k"], x1["q"]],
                      [("final", hh["k"], bias_t["k"]),
                       ("final", hh["q"], bias_t["k"])], "p3")

        with tc.tile_pool(name="psqk", bufs=1, space="PSUM") as qkp:
            # ---- QK + exp (overlaps P4 convs) ----
            for oh in range(2):
                for hp in range(4):
                    pst = qkp.tile([128, BL // 2, 64], F32, tag="pst",
                                   name=f"pst{oh}{hp}", bufs=2)
                    for b2 in range(BL // 2):
                        for par in range(2):
                            b = 2 * b2 + par
                            nc.tensor.matmul(
                                pst[64 * par:64 * par + 64, b2, :],
                                kh[oh][hp * 32:(hp + 1) * 32, b, :],
                                qh[oh][hp * 32:(hp + 1) * 32, b, :],
                                start=True, stop=True,
                                tile_position=(32 * hp, 64 * par))
                    nc.scalar.activation(E_t[:, oh, hp, :, :], pst[:],
                                         AF.Exp, scale=RS)

            with tc.tile_pool(name="pstail", bufs=1, space="PSUM") as tlp:
                # per-oh attention block, interleaved into P5's conv waves
                def attn_block(oh):
                    # phase-major: each phase pipelines across b2
                    for b2 in range(BL // 2):
                        pvt = tlp.tile([128, 128], BF16, tag="ptr",
                                       name=f"pvt{b2}{oh}", bufs=2)
                        nc.tensor.transpose(
                            pvt[:], vh[oh][:, 2 * b2:2 * b2 + 2, :], ident_t[:])
                        nc.vector.tensor_copy(
                            VT[:, b2, oh * 4:(oh + 1) * 4, 0:32],
                            pvt[:].rearrange("k (h d) -> k h d", h=4))
                    psos = []
                    for b2 in range(BL // 2):
                        pso = tlp.tile([128, 4, 33], F32, tag="pso",
                                       name=f"pso{b2}{oh}", bufs=2)
                        psos.append(pso)
                        for hp in range(4):
                            for par in range(2):
                                nc.tensor.matmul(
                                    pso[64 * par:64 * par + 64, hp, :],
                                    E_t[64 * par:64 * par + 64, oh, hp, b2, :],
                                    VT[64 * par:64 * par + 64, b2, oh * 4 + hp, :],
                                    start=True, stop=True)
                        nc.vector.reciprocal(
                            rcp[:, b2, oh * 4:(oh + 1) * 4],
                            pso[:, :, 32:33].rearrange("q h one -> q (h one)"))
                        for hp in range(4):
                            h = oh * 4 + hp
                            dst = OA[:, b2, oh, hp * 32:(hp + 1) * 32]
                            src = pso[:, hp, 0:32]
                            nc.scalar.activation(dst, src, AF.Copy,
                                                 scale=rcp[:, b2, h:h + 1])
                    for b2 in range(BL // 2):
                        pot = tlp.tile([128, 128], BF16, tag="ptr",
                                       name=f"pot{b2}{oh}", bufs=2)
                        nc.tensor.transpose(pot[:], OA[:, b2, oh, :], ident_t[:])
                        nc.vector.tensor_copy(
                            concat[oh][:, 2 * b2:2 * b2 + 2, :],
                            pot[:].rearrange("c (b q) -> c b q", b=2))

                # P4: v conv1;  P5: v conv2 (attn tail interleaved per wave)
                conv_pass(tlp, w8["v"], [x8["v"]],
                          [("relu", x1["v"], bias_t["v"])], "p4", bufs=2)
                conv_pass(tlp, w8["v"], [x1["v"]],
                          [("final", hh["v"], bias_t["v"])], "p5", bufs=2,
                          wave_cb=attn_block)

            with tc.tile_pool(name="psprj", bufs=1, space="PSUM") as prp:
                # output projection: per 128-col block of (b, pix)
                for blk in range(8):
                    pspr = prp.tile([128, 256], F32, tag="pspr",
                                    name=f"pspr{blk}", bufs=2)
                    for oh in range(2):
                        cs = concat[oh][:].rearrange("c b p -> c (b p)")
                        nc.tensor.matmul(
                            pspr[:], cs[:, blk * 128:(blk + 1) * 128],
                            wo_tt[:, oh, :], start=(oh == 0), stop=False)
                    nc.tensor.matmul(pspr[:], ones_t[:], bo_t[:],
                                     start=False, stop=True)
                    nc.vector.tensor_copy(out_sb[:, blk, :], pspr[:])
                    nc.sync.dma_start(dt["out"][blk], out_sb[:, blk, :])
    nc.compile()
    return nc


# ---------------------------------------------------------------------------
# Host-side prep
# ---------------------------------------------------------------------------
def _to_fp8(a):
    return np.clip(np.asarray(a, np.float32), -240.0, 240.0).astype(
        ml_dtypes.float8_e4m3)


def prep_static(wk, bk, wq, bq, wv, bv, wo, bo):
    st = {}
    for nm, w, b in (("q", wq, bq), ("k", wk, bk), ("v", wv, bv)):
        w = np.asarray(w, np.float32)
        # [128c_lo-part, tap, 2(c-half), 256o] fp8, scaled x256
        wt = np.empty((128, NTAP, 2, 256), ml_dtypes.float8_e4m3)
        for ti, (sr, sc, *_r) in enumerate(TAPS):
            wtap = w[:, :, sr + 7, sc + 7].T * WS     # [c, o]
            wt[:, ti] = _to_fp8(wtap.reshape(2, 128, 256).transpose(1, 0, 2))
        st[f"w{nm}"] = np.ascontiguousarray(wt)
        st[f"bias{nm}"] = np.ascontiguousarray(
            np.asarray(b, np.float32).reshape(2, 128).T)
    st["wo_t"] = np.ascontiguousarray(
        np.asarray(wo, np.float32).T.reshape(2, 128, 256).transpose(1, 0, 2)
        .astype(ml_dtypes.bfloat16))
    st["bo"] = np.asarray(bo, np.float32).reshape(1, 256)
    st["ones"] = np.ones((1, 128), np.float32)
    st["ident"] = np.eye(128, dtype=ml_dtypes.bfloat16)
    return st


def prep_core_x(x, core):
    """x: [B, 8, 8, D] -> [128, 2, 8, 8, BL] fp8 (c_lo, c_half, pr, pc, b)."""
    xs = np.asarray(x[core * BL:(core + 1) * BL], np.float32)
    xs = xs.transpose(3, 1, 2, 0).reshape(2, 128, 8, 8, BL).transpose(
        1, 0, 2, 3, 4)
    return np.ascontiguousarray(_to_fp8(xs))


def make_in_maps(q, k, v, st):
    in_maps = []
    for core in range(NCORES):
        m = dict(st)
        m["xq"] = prep_core_x(q, core)
        m["xk"] = prep_core_x(k, core)
        m["xv"] = prep_core_x(v, core)
        in_maps.append(m)
    return in_maps


def gather_out(results):
    outs = [r["out"].reshape(BL, 8, 8, D) for r in results]
    return np.concatenate(outs, axis=0)


_NC_CACHE = None


def _get_nc():
    global _NC_CACHE
    if _NC_CACHE is None:
        _NC_CACHE = build_kernel()
    return _NC_CACHE


def kernel(q, k, v, wk, bk, wq, bq, wv, bv, wo, bo):
    nc = _get_nc()
    st = prep_static(wk, bk, wq, bq, wv, bv, wo, bo)
    in_maps = make_in_maps(np.asarray(q), np.asarray(k), np.asarray(v), st)
    res = bass_utils.run_bass_kernel_spmd(
        nc, in_maps, core_ids=list(range(NCORES)))
    return gather_out(res.results)


# revision 11
# speedup vs baseline: 1.1643x; 1.0659x over previous
"""MultiHeadDoubleAttention on TRN2 — fp8 DoubleRow conv + bf16 attention.

Data-parallel over batch: 8 cores x 16 batch each.

Conv: 15x15 masked conv on 8x8 grid = 65 shift taps of channel matmuls.
All conv matmuls run fp8-e4m3 with perf_mode=DoubleRow: the 256-channel
contraction is packed 2-per-PE-cell ([c, 2, .] operands), so one matmul
per (tap, o-half, psum-bank piece) does the full input-channel reduction
at ~N/2.4ns. Weights are scaled x256 into e4m3 range; copy-out rescales
by 1/256 (end-to-end rel err ~6e-3, tolerance 2e-2 — the output norm is
dominated by the bo bias, which dilutes conv-path error ~5x).

Attention is bf16: QK with 4x2 tile-position packing, one batched exp
per (oh,hp) psum bank, paired 128x128 PE transposes for vh and attn-out,
AV output packed to partition offset 64*par so all normalize/copy ops
stay lane-local.
"""
import sys
sys.path.insert(0, '/opt/trn_rl_repo')
import numpy as np
import ml_dtypes

import concourse.bass as bass
import concourse.bacc as bacc
import concourse.mybir as mybir
import concourse.bass_utils as bass_utils
from concourse.tile import TileContext

F32 = mybir.dt.float32
F32R = mybir.dt.float32r
BF16 = mybir.dt.bfloat16
FP8 = mybir.dt.float8e4
DR = mybir.MatmulPerfMode.DoubleRow
AF = mybir.ActivationFunctionType

B, D, H, DK = 128, 256, 8, 32
NCORES = 8
BL = B // NCORES          # 16 batch per core
NPIX = 64
WS = 256.0                # fp8 weight scale
RS = 1.0 / np.sqrt(DK)


def hollow_mask():
    m = np.ones((15, 15), np.float32)
    for c in range(5):
        m[1 + c:7, c] = 0; m[8:14 - c, c] = 0
        m[c, 1 + c:7] = 0; m[c, 8:14 - c] = 0
        m[1 + c:7, 14 - c] = 0; m[8:14 - c, 14 - c] = 0
        m[14 - c, 1 + c:7] = 0; m[14 - c, 8:14 - c] = 0
    return m


def tap_schedule():
    """All 65 unmasked taps as (sr, sc, ar0, hr, ac0, wc), biggest first."""
    m = hollow_mask()
    taps = []
    for di in range(15):
        for dj in range(15):
            if not m[di, dj]:
                continue
            sr, sc = di - 7, dj - 7
            ar0, ar1 = max(0, sr), min(7, 7 + sr)
            ac0, ac1 = max(0, sc), min(7, 7 + sc)
            taps.append((sr, sc, ar0, ar1 - ar0 + 1, ac0, ac1 - ac0 + 1))
    taps.sort(key=lambda e: -(e[3] * e[5]))
    return taps


TAPS = tap_schedule()
NTAP = len(TAPS)


def tap_pieces(sr, sc, ar0, hr, ac0, wc):
    """Split a tap's output rect at the ir=4 psum-bank boundary.
    Returns list of (bank, irb, ar0, ph, ic0, ac0, wc)."""
    ir0 = ar0 - sr
    ic0 = ac0 - sc
    pieces = []
    lo, hi = ir0, ir0 + hr
    if lo < 4:
        ph = min(hi, 4) - lo
        pieces.append((0, lo, lo + sr, ph, ic0, ac0, wc))
    if hi > 4:
        p0 = max(lo, 4)
        ph = hi - p0
        pieces.append((1, p0 - 4, p0 + sr, ph, ic0, ac0, wc))
    return pieces


PIECES = [tap_pieces(*t) for t in TAPS]
BANK_TOTALS = {0: 0, 1: 0}
for pl in PIECES:
    for p in pl:
        BANK_TOTALS[p[0]] += 1


def build_kernel():
    nc = bacc.Bacc("TRN2", target_bir_lowering=False, debug=False,
                   num_devices=NCORES)
    dt = {}
    for nm in ("q", "k", "v"):
        dt[f"x{nm}"] = nc.dram_tensor(f"x{nm}", [128, 2, 8, 8, BL], FP8,
                                      kind="ExternalInput")
        dt[f"w{nm}"] = nc.dram_tensor(f"w{nm}", [128, NTAP, 2, 256], FP8,
                                      kind="ExternalInput")
        dt[f"bias{nm}"] = nc.dram_tensor(f"bias{nm}", [128, 2], F32,
                                         kind="ExternalInput")
    dt["wo_t"] = nc.dram_tensor("wo_t", [128, 2, 256], BF16, kind="ExternalInput")
    dt["bo"] = nc.dram_tensor("bo", [1, 256], F32R, kind="ExternalInput")
    dt["ones"] = nc.dram_tensor("ones", [1, 128], F32R, kind="ExternalInput")
    dt["ident"] = nc.dram_tensor("ident", [128, 128], BF16, kind="ExternalInput")
    dt["out"] = nc.dram_tensor("out", [8, 128, 256], F32, kind="ExternalOutput")

    with TileContext(nc) as tc:
      with tc.tile_pool(name="persist", bufs=1) as pp:
        # ---- input / weight tiles ----
        x8 = {}
        w8 = {}
        bias_t = {}
        for nm in ("q", "k", "v"):
            x8[nm] = pp.tile([128, 2, 8, 8, BL], FP8, name=f"x{nm}")
            w8[nm] = pp.tile([128, NTAP, 2, 256], FP8, name=f"w{nm}")
            bias_t[nm] = pp.tile([128, 2], F32, name=f"bias{nm}_t")
        x1 = {nm: pp.tile([128, 2, 8, 8, BL], FP8, name=f"x1{nm}")
              for nm in ("q", "k", "v")}
        # conv2 outputs, bf16 [o-half 128, b, pix]
        hh = {nm: [pp.tile([128, BL, NPIX], BF16, name=f"h{nm}{h}")
                   for h in range(2)] for nm in ("q", "k", "v")}
        wo_tt = pp.tile([128, 2, 256], BF16, name="wo_tt")
        bo_t = pp.tile([1, 256], F32R, name="bo_t")
        ones_t = pp.tile([1, 128], F32R, name="ones_t")
        ident_t = pp.tile([128, 128], BF16, name="ident_t")

        # ---- DMAs (x first, then weights in chunks; wq first for P1) ----
        # tiny persist tensors first — P1's copy-out needs bias_q, so these
        # must not queue behind the 13.5MB weight stream
        for nm in ("q", "k", "v"):
            nc.sync.dma_start(bias_t[nm][:], dt[f"bias{nm}"].ap())
        nc.sync.dma_start(wo_tt[:], dt["wo_t"].ap())
        nc.sync.dma_start(bo_t[:], dt["bo"].ap())
        nc.sync.dma_start(ones_t[:], dt["ones"].ap())
        nc.sync.dma_start(ident_t[:], dt["ident"].ap())
        nc.sync.dma_start(x8["q"][:], dt["xq"].ap())
        def load_w(nm):
            t0 = 0
            while t0 < NTAP:
                n = min(2 if t0 == 0 else 8, NTAP - t0)
                nc.sync.dma_start(
                    w8[nm][:, t0:t0 + n],
                    dt[f"w{nm}"].ap()[:, t0:t0 + n])
                t0 += n
        load_w("q")
        nc.sync.dma_start(x8["k"][:], dt["xk"].ap())
        load_w("k")
        nc.sync.dma_start(x8["v"][:], dt["xv"].ap())
        load_w("v")

        # ---- conv pass: fp8 DoubleRow, per-oh waves ----
        def conv_pass(psp, wt, inputs, outs, tag, bufs=4, wave_cb=None):
            """wt: weight tile; inputs: list of x8-like tiles;
            outs: list of (kind, dest, bias) per input:
              kind 'relu' -> dest x1 tile (fp8), kind 'final' -> dest hh pair
            """
            for oh in range(2):
                ps = [[psp.tile([128, 4, 8, BL], F32, tag="cv",
                                name=f"{tag}ps{ii}{oh}{bk}", bufs=bufs)
                       for bk in range(2)] for ii in range(len(inputs))]
                done = {}
                for ti in range(NTAP):
                    lhsT = wt[:, ti, :, oh * 128:(oh + 1) * 128]
                    for ii, xt in enumerate(inputs):
                        for (bk, irb, ar0, ph, ic0, ac0, wc) in PIECES[ti]:
                            cnt = done.get((ii, bk), 0)
                            done[(ii, bk)] = cnt + 1
                            rhs = xt[:, :, ar0:ar0 + ph, ac0:ac0 + wc, :]
                            out = ps[ii][bk][:, irb:irb + ph, ic0:ic0 + wc, :]
                            nc.tensor.matmul(
                                out, lhsT, rhs,
                                start=(cnt == 0),
                                stop=(cnt == BANK_TOTALS[bk] - 1),
                                perf_mode=DR)
                for ii, (kind, dest, bias) in enumerate(outs):
                    for bk in range(2):
                        if kind == "relu":
                            nc.scalar.activation(
                                dest[:, oh, bk * 4:(bk + 1) * 4, :, :],
                                ps[ii][bk][:], AF.Relu,
                                bias=bias[:, oh:oh + 1], scale=1.0 / WS)
                        else:
                            nc.scalar.activation(
                                dest[oh][:, :, bk * 32:(bk + 1) * 32],
                                ps[ii][bk][:].rearrange("c pr pc b -> c b (pr pc)"),
                                AF.Identity,
                                bias=bias[:, oh:oh + 1], scale=1.0 / WS)
                if wave_cb is not None:
                    wave_cb(oh)

        kh, qh, vh = hh["k"], hh["q"], hh["v"]
        # E_t: [128=(par,64k), oh, hp, b2, 64q] bf16
        E_t = pp.tile([128, 2, 4, BL // 2, NPIX], BF16, name="E_t")
        # VT: [128=(par,64k), b2, h, 33] bf16, col 32 = ones
        VT = pp.tile([128, BL // 2, H, 33], BF16, name="VT")
        nc.vector.memset(VT[:, :, :, 32:33], 1.0)
        # OA: [128=(par,64q), b2, oh, 128c] bf16 (normalized attn out)
        OA = pp.tile([128, BL // 2, 2, 128], BF16, name="OA")
        rcp = pp.tile([128, BL // 2, H], F32, name="rcp")
        concat = [pp.tile([128, BL, NPIX], BF16, name=f"concat{h}")
                  for h in range(2)]
        out_sb = pp.tile([128, 8, 256], F32, name="out_sb")

        with tc.tile_pool(name="pscv", bufs=1, space="PSUM") as cvp:
            # P1: q conv1;  P2: k conv1;  P3: k/q conv2 (wk);
            conv_pass(cvp, w8["q"], [x8["q"]], [("relu", x1["q"], bias_t["q"])], "p1")
            conv_pass(cvp, w8["k"], [x8["k"]], [("relu", x1["k"], bias_t["k"])], "p2")
            conv_pass(cvp, w8["k"], [x1["k"], x1["q"]],
                      [("final", hh["k"], bias_t["k"]),
                       ("final", hh["q"], bias_t["k"])], "p3")

            with tc.tile_pool(name="psqk", bufs=1, space="PSUM") as qkp:
                # ---- QK + exp (overlaps P4/P5 convs) ----
                for oh in range(2):
                    for hp in range(4):
                        pst = qkp.tile([128, BL // 2, 64], F32, tag="pst",
                                       name=f"pst{oh}{hp}", bufs=4)
                        for b2 in range(BL // 2):
                            for par in range(2):
                                b = 2 * b2 + par
                                nc.tensor.matmul(
                                    pst[64 * par:64 * par + 64, b2, :],
                                    kh[oh][hp * 32:(hp + 1) * 32, b, :],
                                    qh[oh][hp * 32:(hp + 1) * 32, b, :],
                                    start=True, stop=True,
                                    tile_position=(32 * hp, 64 * par))
                        nc.scalar.activation(E_t[:, oh, hp, :, :], pst[:],
                                             AF.Exp, scale=RS)

                # P4: v conv1;  P5: v conv2
                conv_pass(cvp, w8["v"], [x8["v"]], [("relu", x1["v"], bias_t["v"])], "p4")
                conv_pass(cvp, w8["v"], [x1["v"]], [("final", hh["v"], bias_t["v"])], "p5")

        # ---- attention tail ----
        with tc.tile_pool(name="pstail", bufs=1, space="PSUM") as psp:
            # vh transposes: per (b2, oh): [128, 2b x 64pix] -> [(par,pix), o]
            for b2 in range(BL // 2):
                for oh in range(2):
                    pvt = psp.tile([128, 128], BF16, tag="ptr",
                                   name=f"pvt{b2}{oh}", bufs=2)
                    nc.tensor.transpose(
                        pvt[:], vh[oh][:, 2 * b2:2 * b2 + 2, :], ident_t[:])
                    nc.vector.tensor_copy(
                        VT[:, b2, oh * 4:(oh + 1) * 4, 0:32],
                        pvt[:].rearrange("k (h d) -> k h d", h=4))

            # AV: per (b2, par, h): E.T @ [vh | ones], 8 head slots per tile
            for b2 in range(BL // 2):
                pso = psp.tile([128, H, 33], F32, tag="pso",
                               name=f"pso{b2}", bufs=4)
                for oh in range(2):
                    for hp in range(4):
                        for par in range(2):
                            nc.tensor.matmul(
                                pso[64 * par:64 * par + 64, oh * 4 + hp, :],
                                E_t[64 * par:64 * par + 64, oh, hp, b2, :],
                                VT[64 * par:64 * par + 64, b2, oh * 4 + hp, :],
                                start=True, stop=True)
                nc.vector.reciprocal(
                    rcp[:, b2, :],
                    pso[:, :, 32:33].rearrange("q h one -> q (h one)"))
                for h in range(H):
                    oh, hp = h // 4, h % 4
                    dst = OA[:, b2, oh, hp * 32:(hp + 1) * 32]
                    src = pso[:, h, 0:32]
                    if h % 2 == 0:
                        nc.scalar.activation(dst, src, AF.Copy,
                                             scale=rcp[:, b2, h:h + 1])
                    else:
                        nc.vector.tensor_scalar_mul(dst, src,
                                                    rcp[:, b2, h:h + 1])

            # attn-out transposes: [(par,q), c] -> [c, (par,q)] -> concat
            for b2 in range(BL // 2):
                for oh in range(2):
                    pot = psp.tile([128, 128], BF16, tag="ptr",
                                   name=f"pot{b2}{oh}", bufs=2)
                    nc.tensor.transpose(pot[:], OA[:, b2, oh, :], ident_t[:])
                    nc.vector.tensor_copy(
                        concat[oh][:, 2 * b2:2 * b2 + 2, :],
                        pot[:].rearrange("c (b q) -> c b q", b=2))

            # output projection: per 128-col block of (b, pix)
            for blk in range(8):
                pspr = psp.tile([128, 256], F32, tag="pspr",
                                name=f"pspr{blk}", bufs=2)
                for oh in range(2):
                    cs = concat[oh][:].rearrange("c b p -> c (b p)")
                    nc.tensor.matmul(
                        pspr[:], cs[:, blk * 128:(blk + 1) * 128],
                        wo_tt[:, oh, :], start=(oh == 0), stop=False)
                nc.tensor.matmul(pspr[:], ones_t[:], bo_t[:],
                                 start=False, stop=True)
                if blk % 2 == 0:
                    nc.vector.tensor_copy(out_sb[:, blk, :], pspr[:])
                else:
                    nc.scalar.copy(out_sb[:, blk, :], pspr[:])
                nc.sync.dma_start(dt["out"][blk], out_sb[:, blk, :])
    nc.compile()
    return nc


# ---------------------------------------------------------------------------
# Host-side prep
# ---------------------------------------------------------------------------
def _to_fp8(a):
    return np.clip(np.asarray(a, np.float32), -240.0, 240.0).astype(
        ml_dtypes.float8_e4m3)


def prep_static(wk, bk, wq, bq, wv, bv, wo, bo):
    st = {}
    for nm, w, b in (("q", wq, bq), ("k", wk, bk), ("v", wv, bv)):
        w = np.asarray(w, np.float32)
        # [128c_lo-part, tap, 2(c-half), 256o] fp8, scaled x256
        wt = np.empty((128, NTAP, 2, 256), ml_dtypes.float8_e4m3)
        for ti, (sr, sc, *_r) in enumerate(TAPS):
            wtap = w[:, :, sr + 7, sc + 7].T * WS     # [c, o]
            wt[:, ti] = _to_fp8(wtap.reshape(2, 128, 256).transpose(1, 0, 2))
        st[f"w{nm}"] = np.ascontiguousarray(wt)
        st[f"bias{nm}"] = np.ascontiguousarray(
            np.asarray(b, np.float32).reshape(2, 128).T)
    st["wo_t"] = np.ascontiguousarray(
        np.asarray(wo, np.float32).T.reshape(2, 128, 256).transpose(1, 0, 2)
        .astype(ml_dtypes.bfloat16))
    st["bo"] = np.asarray(bo, np.float32).reshape(1, 256)
    st["ones"] = np.ones((1, 128), np.float32)
    st["ident"] = np.eye(128, dtype=ml_dtypes.bfloat16)
    return st


def prep_core_x(x, core):
    """x: [B, 8, 8, D] -> [128, 2, 8, 8, BL] fp8 (c_lo, c_half, pr, pc, b)."""
    xs = np.asarray(x[core * BL:(core + 1) * BL], np.float32)
    xs = xs.transpose(3, 1, 2, 0).reshape(2, 128, 8, 8, BL).transpose(
        1, 0, 2, 3, 4)
    return np.ascontiguousarray(_to_fp8(xs))


def make_in_maps(q, k, v, st):
    in_maps = []
    for core in range(NCORES):
        m = dict(st)
        m["xq"] = prep_core_x(q, core)
        m["xk"] = prep_core_x(k, core)
        m["xv"] = prep_core_x(v, core)
        in_maps.append(m)
    return in_maps


def gather_out(results):
    outs = [r["out"].reshape(BL, 8, 8, D) for r in results]
    return np.concatenate(outs, axis=0)


_NC_CACHE = None


def _get_nc():
    global _NC_CACHE
    if _NC_CACHE is None:
        _NC_CACHE = build_kernel()
    return _NC_CACHE


def kernel(q, k, v, wk, bk, wq, bq, wv, bv, wo, bo):
    nc = _get_nc()
    st = prep_static(wk, bk, wq, bq, wv, bv, wo, bo)
    in_maps = make_in_maps(np.asarray(q), np.asarray(k), np.asarray(v), st)
    res = bass_utils.run_bass_kernel_spmd(
        nc, in_maps, core_ids=list(range(NCORES)))
    return gather_out(res.results)


# revision 12
# speedup vs baseline: 1.1645x; 1.0002x over previous
"""MultiHeadDoubleAttention on TRN2 — fp8 DoubleRow conv + bf16 attention.

Data-parallel over batch: 8 cores x 16 batch each.

Conv: 15x15 masked conv on 8x8 grid = 65 shift taps of channel matmuls.
All conv matmuls run fp8-e4m3 with perf_mode=DoubleRow: the 256-channel
contraction is packed 2-per-PE-cell ([c, 2, .] operands), so one matmul
per (tap, o-half, psum-bank piece) does the full input-channel reduction
at ~N/2.4ns. Weights are scaled x256 into e4m3 range; copy-out rescales
by 1/256 (end-to-end rel err ~6e-3, tolerance 2e-2 — the output norm is
dominated by the bo bias, which dilutes conv-path error ~5x).

Attention is bf16: QK with 4x2 tile-position packing, one batched exp
per (oh,hp) psum bank, paired 128x128 PE transposes for vh and attn-out,
AV output packed to partition offset 64*par so all normalize/copy ops
stay lane-local.
"""
import sys
sys.path.insert(0, '/opt/trn_rl_repo')
import numpy as np
import ml_dtypes

import concourse.bass as bass
import concourse.bacc as bacc
import concourse.mybir as mybir
import concourse.bass_utils as bass_utils
from concourse.tile import TileContext

F32 = mybir.dt.float32
F32R = mybir.dt.float32r
BF16 = mybir.dt.bfloat16
FP8 = mybir.dt.float8e4
DR = mybir.MatmulPerfMode.DoubleRow
AF = mybir.ActivationFunctionType

B, D, H, DK = 128, 256, 8, 32
NCORES = 8
BL = B // NCORES          # 16 batch per core
NPIX = 64
WS = 64.0                 # fp8 weight scale; x1 stored x64
RS = 1.0 / np.sqrt(DK)


def hollow_mask():
    m = np.ones((15, 15), np.float32)
    for c in range(5):
        m[1 + c:7, c] = 0; m[8:14 - c, c] = 0
        m[c, 1 + c:7] = 0; m[c, 8:14 - c] = 0
        m[1 + c:7, 14 - c] = 0; m[8:14 - c, 14 - c] = 0
        m[14 - c, 1 + c:7] = 0; m[14 - c, 8:14 - c] = 0
    return m


def tap_schedule():
    """All 65 unmasked taps as (sr, sc, ar0, hr, ac0, wc), biggest first."""
    m = hollow_mask()
    taps = []
    for di in range(15):
        for dj in range(15):
            if not m[di, dj]:
                continue
            sr, sc = di - 7, dj - 7
            ar0, ar1 = max(0, sr), min(7, 7 + sr)
            ac0, ac1 = max(0, sc), min(7, 7 + sc)
            taps.append((sr, sc, ar0, ar1 - ar0 + 1, ac0, ac1 - ac0 + 1))
    taps.sort(key=lambda e: -(e[3] * e[5]))
    return taps


TAPS = tap_schedule()
NTAP = len(TAPS)


def tap_pieces(sr, sc, ar0, hr, ac0, wc):
    """Split a tap's output rect at the ir=4 psum-bank boundary.
    Returns list of (bank, irb, ar0, ph, ic0, ac0, wc)."""
    ir0 = ar0 - sr
    ic0 = ac0 - sc
    pieces = []
    lo, hi = ir0, ir0 + hr
    if lo < 4:
        ph = min(hi, 4) - lo
        pieces.append((0, lo, lo + sr, ph, ic0, ac0, wc))
    if hi > 4:
        p0 = max(lo, 4)
        ph = hi - p0
        pieces.append((1, p0 - 4, p0 + sr, ph, ic0, ac0, wc))
    return pieces


PIECES = [tap_pieces(*t) for t in TAPS]
BANK_TOTALS = {0: 0, 1: 0}
for pl in PIECES:
    for p in pl:
        BANK_TOTALS[p[0]] += 1


def build_kernel():
    nc = bacc.Bacc("TRN2", target_bir_lowering=False, debug=False,
                   num_devices=NCORES)
    dt = {}
    for nm in ("q", "k", "v"):
        dt[f"x{nm}"] = nc.dram_tensor(f"x{nm}", [128, 2, 8, 8, BL], FP8,
                                      kind="ExternalInput")
        dt[f"w{nm}"] = nc.dram_tensor(f"w{nm}", [128, NTAP, 2, 256], FP8,
                                      kind="ExternalInput")
        dt[f"bias{nm}"] = nc.dram_tensor(f"bias{nm}", [128, 2, 2], F32,
                                         kind="ExternalInput")
    dt["wo_t"] = nc.dram_tensor("wo_t", [128, 2, 256], BF16, kind="ExternalInput")
    dt["bo"] = nc.dram_tensor("bo", [1, 256], F32R, kind="ExternalInput")
    dt["ones"] = nc.dram_tensor("ones", [1, 128], F32R, kind="ExternalInput")
    dt["ident"] = nc.dram_tensor("ident", [128, 128], BF16, kind="ExternalInput")
    dt["out"] = nc.dram_tensor("out", [8, 128, 256], F32, kind="ExternalOutput")

    with TileContext(nc) as tc:
      with tc.tile_pool(name="persist", bufs=1) as pp:
        # ---- input / weight tiles ----
        x8 = {}
        w8 = {}
        bias_t = {}
        for nm in ("q", "k", "v"):
            x8[nm] = pp.tile([128, 2, 8, 8, BL], FP8, name=f"x{nm}")
            w8[nm] = pp.tile([128, NTAP, 2, 256], FP8, name=f"w{nm}")
            bias_t[nm] = pp.tile([128, 2, 2], F32, name=f"bias{nm}_t")
        x1 = {nm: pp.tile([128, 2, 8, 8, BL], FP8, name=f"x1{nm}")
              for nm in ("q", "k", "v")}
        # conv2 outputs, bf16 [o-half 128, b, pix]
        hh = {nm: [pp.tile([128, BL, NPIX], BF16, name=f"h{nm}{h}")
                   for h in range(2)] for nm in ("q", "k", "v")}
        wo_tt = pp.tile([128, 2, 256], BF16, name="wo_tt")
        bo_t = pp.tile([1, 256], F32R, name="bo_t")
        ones_t = pp.tile([1, 128], F32R, name="ones_t")
        ident_t = pp.tile([128, 128], BF16, name="ident_t")

        # ---- DMAs (x first, then weights in chunks; wq first for P1) ----
        def load_w(nm, t0=0, tend=None):
            tend = NTAP if tend is None else tend
            while t0 < tend:
                n = min(2 if t0 == 0 else 8, tend - t0)
                nc.sync.dma_start(
                    w8[nm][:, t0:t0 + n],
                    dt[f"w{nm}"].ap()[:, t0:t0 + n])
                t0 += n
        # critical path first: xq + first wq taps, then tiny persists
        # (bias gates P1's copy-out ~20us later), then the weight streams
        nc.sync.dma_start(x8["q"][:], dt["xq"].ap())
        load_w("q", 0, 10)
        for nm in ("q", "k", "v"):
            nc.sync.dma_start(bias_t[nm][:], dt[f"bias{nm}"].ap())
        nc.sync.dma_start(wo_tt[:], dt["wo_t"].ap())
        nc.sync.dma_start(bo_t[:], dt["bo"].ap())
        nc.sync.dma_start(ones_t[:], dt["ones"].ap())
        nc.sync.dma_start(ident_t[:], dt["ident"].ap())
        load_w("q", 10)
        nc.sync.dma_start(x8["k"][:], dt["xk"].ap())
        load_w("k")
        nc.sync.dma_start(x8["v"][:], dt["xv"].ap())
        load_w("v")

        # ---- conv pass: fp8 DoubleRow, per-oh waves ----
        def conv_pass(psp, wt, inputs, outs, tag, bufs=4, wave_cb=None):
            """wt: weight tile; inputs: list of x8-like tiles;
            outs: list of (kind, dest, bias) per input:
              kind 'relu' -> dest x1 tile (fp8), kind 'final' -> dest hh pair
            """
            for oh in range(2):
                ps = [[psp.tile([128, 4, 8, BL], F32, tag="cv",
                                name=f"{tag}ps{ii}{oh}{bk}", bufs=bufs)
                       for bk in range(2)] for ii in range(len(inputs))]
                done = {}
                for ti in range(NTAP):
                    lhsT = wt[:, ti, :, oh * 128:(oh + 1) * 128]
                    for ii, xt in enumerate(inputs):
                        for (bk, irb, ar0, ph, ic0, ac0, wc) in PIECES[ti]:
                            cnt = done.get((ii, bk), 0)
                            done[(ii, bk)] = cnt + 1
                            rhs = xt[:, :, ar0:ar0 + ph, ac0:ac0 + wc, :]
                            out = ps[ii][bk][:, irb:irb + ph, ic0:ic0 + wc, :]
                            nc.tensor.matmul(
                                out, lhsT, rhs,
                                start=(cnt == 0),
                                stop=(cnt == BANK_TOTALS[bk] - 1),
                                perf_mode=DR)
                for ii, (kind, dest, bias) in enumerate(outs):
                    for bk in range(2):
                        if kind == "relu":
                            # psum = WS*conv; x1 = relu(psum + WS*b)  (x64)
                            if bk == 0:
                                nc.scalar.activation(
                                    dest[:, oh, bk * 4:(bk + 1) * 4, :, :],
                                    ps[ii][bk][:], AF.Relu,
                                    bias=bias[:, 1, oh:oh + 1], scale=1.0)
                            else:
                                nc.vector.tensor_scalar(
                                    dest[:, oh, bk * 4:(bk + 1) * 4, :, :],
                                    ps[ii][bk][:],
                                    bias[:, 1, oh:oh + 1], 0.0,
                                    mybir.AluOpType.add, mybir.AluOpType.max)
                        else:
                            # psum = WS^2*conv; out = psum/WS^2 + b  (bf16)
                            if bk == 0:
                                nc.scalar.activation(
                                    dest[oh][:, :, bk * 32:(bk + 1) * 32],
                                    ps[ii][bk][:].rearrange("c pr pc b -> c b (pr pc)"),
                                    AF.Identity,
                                    bias=bias[:, 0, oh:oh + 1], scale=1.0 / WS**2)
                            else:
                                nc.vector.tensor_scalar(
                                    dest[oh][:, :, bk * 32:(bk + 1) * 32],
                                    ps[ii][bk][:].rearrange("c pr pc b -> c b (pr pc)"),
                                    1.0 / WS**2, bias[:, 0, oh:oh + 1],
                                    mybir.AluOpType.mult, mybir.AluOpType.add)
                if wave_cb is not None:
                    wave_cb(oh)

        kh, qh, vh = hh["k"], hh["q"], hh["v"]
        # E_t: [128=(par,64k), oh, hp, b2, 64q] bf16
        E_t = pp.tile([128, 2, 4, BL // 2, NPIX], BF16, name="E_t")
        # VT: [128=(par,64k), b2, h, 33] bf16, col 32 = ones
        VT = pp.tile([128, BL // 2, H, 33], BF16, name="VT")
        nc.vector.memset(VT[:, :, :, 32:33], 1.0)
        # OA: [128=(par,64q), b2, oh, 128c] bf16 (normalized attn out)
        OA = pp.tile([128, BL // 2, 2, 128], BF16, name="OA")
        rcp = pp.tile([128, BL // 2, H], F32, name="rcp")
        concat = [pp.tile([128, BL, NPIX], BF16, name=f"concat{h}")
                  for h in range(2)]
        out_sb = pp.tile([128, 8, 256], F32, name="out_sb")

        with tc.tile_pool(name="pscv", bufs=1, space="PSUM") as cvp:
            # P1: q conv1;  P2: k conv1;  P3: k/q conv2 (wk);
            conv_pass(cvp, w8["q"], [x8["q"]], [("relu", x1["q"], bias_t["q"])], "p1")
            conv_pass(cvp, w8["k"], [x8["k"]], [("relu", x1["k"], bias_t["k"])], "p2")
            conv_pass(cvp, w8["k"], [x1["k"], x1["q"]],
                      [("final", hh["k"], bias_t["k"]),
                       ("final", hh["q"], bias_t["k"])], "p3")

            with tc.tile_pool(name="psqk", bufs=1, space="PSUM") as qkp:
                # ---- QK + exp (overlaps P4/P5 convs) ----
                for oh in range(2):
                    for hp in range(4):
                        pst = qkp.tile([128, BL // 2, 64], F32, tag="pst",
                                       name=f"pst{oh}{hp}", bufs=4)
                        for b2 in range(BL // 2):
                            for par in range(2):
                                b = 2 * b2 + par
                                nc.tensor.matmul(
                                    pst[64 * par:64 * par + 64, b2, :],
                                    kh[oh][hp * 32:(hp + 1) * 32, b, :],
                                    qh[oh][hp * 32:(hp + 1) * 32, b, :],
                                    start=True, stop=True,
                                    tile_position=(32 * hp, 64 * par))
                        nc.scalar.activation(E_t[:, oh, hp, :, :], pst[:],
                                             AF.Exp, scale=RS)

                # P4: v conv1;  P5: v conv2
                conv_pass(cvp, w8["v"], [x8["v"]], [("relu", x1["v"], bias_t["v"])], "p4")
                conv_pass(cvp, w8["v"], [x1["v"]], [("final", hh["v"], bias_t["v"])], "p5")

        # ---- attention tail ----
        with tc.tile_pool(name="pstail", bufs=1, space="PSUM") as psp:
            # vh transposes: per (b2, oh): [128, 2b x 64pix] -> [(par,pix), o]
            for b2 in range(BL // 2):
                for oh in range(2):
                    pvt = psp.tile([128, 128], BF16, tag="ptr",
                                   name=f"pvt{b2}{oh}", bufs=2)
                    nc.tensor.transpose(
                        pvt[:], vh[oh][:, 2 * b2:2 * b2 + 2, :], ident_t[:])
                    nc.vector.tensor_copy(
                        VT[:, b2, oh * 4:(oh + 1) * 4, 0:32],
                        pvt[:].rearrange("k (h d) -> k h d", h=4))

            # AV: per (b2, par, h): E.T @ [vh | ones], 8 head slots per tile
            for b2 in range(BL // 2):
                pso = psp.tile([128, H, 33], F32, tag="pso",
                               name=f"pso{b2}", bufs=4)
                for oh in range(2):
                    for hp in range(4):
                        for par in range(2):
                            nc.tensor.matmul(
                                pso[64 * par:64 * par + 64, oh * 4 + hp, :],
                                E_t[64 * par:64 * par + 64, oh, hp, b2, :],
                                VT[64 * par:64 * par + 64, b2, oh * 4 + hp, :],
                                start=True, stop=True)
                nc.vector.reciprocal(
                    rcp[:, b2, :],
                    pso[:, :, 32:33].rearrange("q h one -> q (h one)"))
                for h in range(H):
                    oh, hp = h // 4, h % 4
                    dst = OA[:, b2, oh, hp * 32:(hp + 1) * 32]
                    src = pso[:, h, 0:32]
                    if h % 2 == 0:
                        nc.scalar.activation(dst, src, AF.Copy,
                                             scale=rcp[:, b2, h:h + 1])
                    else:
                        nc.vector.tensor_scalar_mul(dst, src,
                                                    rcp[:, b2, h:h + 1])

            # attn-out transposes: [(par,q), c] -> [c, (par,q)] -> concat
            for b2 in range(BL // 2):
                for oh in range(2):
                    pot = psp.tile([128, 128], BF16, tag="ptr",
                                   name=f"pot{b2}{oh}", bufs=2)
                    nc.tensor.transpose(pot[:], OA[:, b2, oh, :], ident_t[:])
                    nc.vector.tensor_copy(
                        concat[oh][:, 2 * b2:2 * b2 + 2, :],
                        pot[:].rearrange("c (b q) -> c b q", b=2))

            # output projection: per 128-col block of (b, pix)
            for blk in range(8):
                pspr = psp.tile([128, 256], F32, tag="pspr",
                                name=f"pspr{blk}", bufs=2)
                for oh in range(2):
                    cs = concat[oh][:].rearrange("c b p -> c (b p)")
                    nc.tensor.matmul(
                        pspr[:], cs[:, blk * 128:(blk + 1) * 128],
                        wo_tt[:, oh, :], start=(oh == 0), stop=False)
                nc.tensor.matmul(pspr[:], ones_t[:], bo_t[:],
                                 start=False, stop=True)
                if blk % 2 == 0:
                    nc.vector.tensor_copy(out_sb[:, blk, :], pspr[:])
                else:
                    nc.scalar.copy(out_sb[:, blk, :], pspr[:])
                nc.sync.dma_start(dt["out"][blk], out_sb[:, blk, :])
    nc.compile()
    return nc


# ---------------------------------------------------------------------------
# Host-side prep
# ---------------------------------------------------------------------------
def _to_fp8(a):
    return np.clip(np.asarray(a, np.float32), -240.0, 240.0).astype(
        ml_dtypes.float8_e4m3)


def prep_static(wk, bk, wq, bq, wv, bv, wo, bo):
    st = {}
    for nm, w, b in (("q", wq, bq), ("k", wk, bk), ("v", wv, bv)):
        w = np.asarray(w, np.float32)
        # [128c_lo-part, tap, 2(c-half), 256o] fp8, scaled x256
        wt = np.empty((128, NTAP, 2, 256), ml_dtypes.float8_e4m3)
        for ti, (sr, sc, *_r) in enumerate(TAPS):
            wtap = w[:, :, sr + 7, sc + 7].T * WS     # [c, o]
            wt[:, ti] = _to_fp8(wtap.reshape(2, 128, 256).transpose(1, 0, 2))
        st[f"w{nm}"] = np.ascontiguousarray(wt)
        bb = np.asarray(b, np.float32).reshape(2, 128).T  # [128c, 2oh]
        st[f"bias{nm}"] = np.ascontiguousarray(
            np.stack([bb, WS * bb], axis=1))  # [128, 2kind, 2oh]
    st["wo_t"] = np.ascontiguousarray(
        np.asarray(wo, np.float32).T.reshape(2, 128, 256).transpose(1, 0, 2)
        .astype(ml_dtypes.bfloat16))
    st["bo"] = np.asarray(bo, np.float32).reshape(1, 256)
    st["ones"] = np.ones((1, 128), np.float32)
    st["ident"] = np.eye(128, dtype=ml_dtypes.bfloat16)
    return st


def prep_core_x(x, core):
    """x: [B, 8, 8, D] -> [128, 2, 8, 8, BL] fp8 (c_lo, c_half, pr, pc, b)."""
    xs = np.asarray(x[core * BL:(core + 1) * BL], np.float32)
    xs = xs.transpose(3, 1, 2, 0).reshape(2, 128, 8, 8, BL).transpose(
        1, 0, 2, 3, 4)
    return np.ascontiguousarray(_to_fp8(xs))


def make_in_maps(q, k, v, st):
    in_maps = []
    for core in range(NCORES):
        m = dict(st)
        m["xq"] = prep_core_x(q, core)
        m["xk"] = prep_core_x(k, core)
        m["xv"] = prep_core_x(v, core)
        in_maps.append(m)
    return in_maps


def gather_out(results):
    outs = [r["out"].reshape(BL, 8, 8, D) for r in results]
    return np.concatenate(outs, axis=0)


_NC_CACHE = None


def _get_nc():
    global _NC_CACHE
    if _NC_CACHE is None:
        _NC_CACHE = build_kernel()
    return _NC_CACHE


def kernel(q, k, v, wk, bk, wq, bq, wv, bv, wo, bo):
    nc = _get_nc()
    st = prep_static(wk, bk, wq, bq, wv, bv, wo, bo)
    in_maps = make_in_maps(np.asarray(q), np.asarray(k), np.asarray(v), st)
    res = bass_utils.run_bass_kernel_spmd(
        nc, in_maps, core_ids=list(range(NCORES)))
    return gather_out(res.results)


# revision 13
# speedup vs baseline: 1.2218x; 1.0492x over previous
"""MultiHeadDoubleAttention on TRN2 — fp8 DoubleRow conv + bf16 attention.

Data-parallel over batch: 8 cores x 16 batch each.

Conv: 15x15 masked conv on 8x8 grid = 65 shift taps of channel matmuls.
All conv matmuls run fp8-e4m3 with perf_mode=DoubleRow: the 256-channel
contraction is packed 2-per-PE-cell ([c, 2, .] operands), so one matmul
per (tap, o-half, psum-bank piece) does the full input-channel reduction
at ~N/2.4ns. Weights are scaled x256 into e4m3 range; copy-out rescales
by 1/256 (end-to-end rel err ~6e-3, tolerance 2e-2 — the output norm is
dominated by the bo bias, which dilutes conv-path error ~5x).

Attention is bf16: QK with 4x2 tile-position packing, one batched exp
per (oh,hp) psum bank, paired 128x128 PE transposes for vh and attn-out,
AV output packed to partition offset 64*par so all normalize/copy ops
stay lane-local.
"""
import sys
sys.path.insert(0, '/opt/trn_rl_repo')
import numpy as np
import ml_dtypes

import concourse.bass as bass
import concourse.bacc as bacc
import concourse.mybir as mybir
import concourse.bass_utils as bass_utils
from concourse.tile import TileContext
import concourse.tile as _tile_mod
import os as _os


def _install_ldw_dedup():
    """Drop InstLdweights whose weights AP matches the immediately
    preceding LDW (consecutive matmuls sharing a stationary operand:
    psum-bank pieces of one tap, P3's two inputs). Matmuls are
    non-self-loading (ldweights=False) and nothing depends on the LDW
    instructions, so removal is safe: the PE keeps the loaded weights.
    """
    if getattr(_tile_mod, "_ldw_dedup_installed", False):
        return
    _orig = _tile_mod.tile_legalize

    def _sig(i):
        return (str(i.ins[0]), str(i.perf_mode), str(i.tile_position),
                str(i.tile_size), str(i.is_transpose))

    def wrapped(ordered, nc):
        out = _orig(ordered, nc)
        if _os.environ.get("LDW_DEDUP", "1") != "1":
            return out
        removed = 0
        for bb, insts in list(out.items()):
            last = None
            keep = []
            for i in insts:
                if isinstance(i, mybir.InstLdweights):
                    sg = _sig(i)
                    if last is not None and sg == last:
                        removed += 1
                        continue
                    last = sg
                keep.append(i)
            out[bb] = keep
        if removed:
            print(f"ldw_dedup: removed {removed} duplicate LDWEIGHTS")
        return out

    _tile_mod.tile_legalize = wrapped
    _tile_mod._ldw_dedup_installed = True


_install_ldw_dedup()

F32 = mybir.dt.float32
F32R = mybir.dt.float32r
BF16 = mybir.dt.bfloat16
FP8 = mybir.dt.float8e4
DR = mybir.MatmulPerfMode.DoubleRow
AF = mybir.ActivationFunctionType

B, D, H, DK = 128, 256, 8, 32
NCORES = 8
BL = B // NCORES          # 16 batch per core
NPIX = 64
WS = 64.0                 # fp8 weight scale; x1 stored x64
RS = 1.0 / np.sqrt(DK)


def hollow_mask():
    m = np.ones((15, 15), np.float32)
    for c in range(5):
        m[1 + c:7, c] = 0; m[8:14 - c, c] = 0
        m[c, 1 + c:7] = 0; m[c, 8:14 - c] = 0
        m[1 + c:7, 14 - c] = 0; m[8:14 - c, 14 - c] = 0
        m[14 - c, 1 + c:7] = 0; m[14 - c, 8:14 - c] = 0
    return m


def tap_schedule():
    """All 65 unmasked taps as (sr, sc, ar0, hr, ac0, wc), biggest first."""
    m = hollow_mask()
    taps = []
    for di in range(15):
        for dj in range(15):
            if not m[di, dj]:
                continue
            sr, sc = di - 7, dj - 7
            ar0, ar1 = max(0, sr), min(7, 7 + sr)
            ac0, ac1 = max(0, sc), min(7, 7 + sc)
            taps.append((sr, sc, ar0, ar1 - ar0 + 1, ac0, ac1 - ac0 + 1))
    taps.sort(key=lambda e: -(e[3] * e[5]))
    return taps


TAPS = tap_schedule()
NTAP = len(TAPS)


def tap_pieces(sr, sc, ar0, hr, ac0, wc):
    """Split a tap's output rect at the ir=4 psum-bank boundary.
    Returns list of (bank, irb, ar0, ph, ic0, ac0, wc)."""
    ir0 = ar0 - sr
    ic0 = ac0 - sc
    pieces = []
    lo, hi = ir0, ir0 + hr
    if lo < 4:
        ph = min(hi, 4) - lo
        pieces.append((0, lo, lo + sr, ph, ic0, ac0, wc))
    if hi > 4:
        p0 = max(lo, 4)
        ph = hi - p0
        pieces.append((1, p0 - 4, p0 + sr, ph, ic0, ac0, wc))
    return pieces


PIECES = [tap_pieces(*t) for t in TAPS]
BANK_TOTALS = {0: 0, 1: 0}
for pl in PIECES:
    for p in pl:
        BANK_TOTALS[p[0]] += 1


def build_kernel():
    nc = bacc.Bacc("TRN2", target_bir_lowering=False, debug=False,
                   num_devices=NCORES)
    dt = {}
    for nm in ("q", "k", "v"):
        dt[f"x{nm}"] = nc.dram_tensor(f"x{nm}", [128, 2, 8, 8, BL], FP8,
                                      kind="ExternalInput")
        dt[f"w{nm}"] = nc.dram_tensor(f"w{nm}", [128, NTAP, 2, 256], FP8,
                                      kind="ExternalInput")
        dt[f"bias{nm}"] = nc.dram_tensor(f"bias{nm}", [128, 2, 2], F32,
                                         kind="ExternalInput")
    dt["wo_t"] = nc.dram_tensor("wo_t", [128, 2, 256], BF16, kind="ExternalInput")
    dt["bo"] = nc.dram_tensor("bo", [1, 256], F32R, kind="ExternalInput")
    dt["ones"] = nc.dram_tensor("ones", [1, 128], F32R, kind="ExternalInput")
    dt["ident"] = nc.dram_tensor("ident", [128, 128], BF16, kind="ExternalInput")
    dt["out"] = nc.dram_tensor("out", [8, 128, 256], F32, kind="ExternalOutput")

    with TileContext(nc) as tc:
      with tc.tile_pool(name="persist", bufs=1) as pp:
        # ---- input / weight tiles ----
        x8 = {}
        w8 = {}
        bias_t = {}
        for nm in ("q", "k", "v"):
            x8[nm] = pp.tile([128, 2, 8, 8, BL], FP8, name=f"x{nm}")
            w8[nm] = pp.tile([128, NTAP, 2, 256], FP8, name=f"w{nm}")
            bias_t[nm] = pp.tile([128, 2, 2], F32, name=f"bias{nm}_t")
        x1 = {nm: pp.tile([128, 2, 8, 8, BL], FP8, name=f"x1{nm}")
              for nm in ("q", "k", "v")}
        # conv2 outputs, bf16 [o-half 128, b, pix]
        hh = {nm: [pp.tile([128, BL, NPIX], BF16, name=f"h{nm}{h}")
                   for h in range(2)] for nm in ("q", "k", "v")}
        wo_tt = pp.tile([128, 2, 256], BF16, name="wo_tt")
        bo_t = pp.tile([1, 256], F32R, name="bo_t")
        ones_t = pp.tile([1, 128], F32R, name="ones_t")
        ident_t = pp.tile([128, 128], BF16, name="ident_t")

        # ---- DMAs (x first, then weights in chunks; wq first for P1) ----
        def load_w(nm, t0=0, tend=None):
            tend = NTAP if tend is None else tend
            while t0 < tend:
                n = min(2 if t0 == 0 else 8, tend - t0)
                nc.sync.dma_start(
                    w8[nm][:, t0:t0 + n],
                    dt[f"w{nm}"].ap()[:, t0:t0 + n])
                t0 += n
        # critical path first: xq + first wq taps, then tiny persists
        # (bias gates P1's copy-out ~20us later), then the weight streams
        nc.sync.dma_start(x8["q"][:], dt["xq"].ap())
        load_w("q", 0, 10)
        for nm in ("q", "k", "v"):
            nc.sync.dma_start(bias_t[nm][:], dt[f"bias{nm}"].ap())
        nc.sync.dma_start(wo_tt[:], dt["wo_t"].ap())
        nc.sync.dma_start(bo_t[:], dt["bo"].ap())
        nc.sync.dma_start(ones_t[:], dt["ones"].ap())
        nc.sync.dma_start(ident_t[:], dt["ident"].ap())
        load_w("q", 10)
        nc.sync.dma_start(x8["k"][:], dt["xk"].ap())
        load_w("k")
        nc.sync.dma_start(x8["v"][:], dt["xv"].ap())
        load_w("v")

        # ---- conv pass: fp8 DoubleRow, per-oh waves ----
        def conv_pass(psp, wt, inputs, outs, tag, bufs=4, wave_cb=None):
            """wt: weight tile; inputs: list of x8-like tiles;
            outs: list of (kind, dest, bias) per input:
              kind 'relu' -> dest x1 tile (fp8), kind 'final' -> dest hh pair
            """
            for oh in range(2):
                ps = [[psp.tile([128, 4, 8, BL], F32, tag="cv",
                                name=f"{tag}ps{ii}{oh}{bk}", bufs=bufs)
                       for bk in range(2)] for ii in range(len(inputs))]
                done = {}
                for ti in range(NTAP):
                    lhsT = wt[:, ti, :, oh * 128:(oh + 1) * 128]
                    for ii, xt in enumerate(inputs):
                        for (bk, irb, ar0, ph, ic0, ac0, wc) in PIECES[ti]:
                            cnt = done.get((ii, bk), 0)
                            done[(ii, bk)] = cnt + 1
                            rhs = xt[:, :, ar0:ar0 + ph, ac0:ac0 + wc, :]
                            out = ps[ii][bk][:, irb:irb + ph, ic0:ic0 + wc, :]
                            nc.tensor.matmul(
                                out, lhsT, rhs,
                                start=(cnt == 0),
                                stop=(cnt == BANK_TOTALS[bk] - 1),
                                perf_mode=DR)
                for ii, (kind, dest, bias) in enumerate(outs):
                    for bk in range(2):
                        if kind == "relu":
                            # psum = WS*conv; x1 = relu(psum + WS*b)  (x64)
                            if bk == 0:
                                nc.scalar.activation(
                                    dest[:, oh, bk * 4:(bk + 1) * 4, :, :],
                                    ps[ii][bk][:], AF.Relu,
                                    bias=bias[:, 1, oh:oh + 1], scale=1.0)
                            else:
                                nc.vector.tensor_scalar(
                                    dest[:, oh, bk * 4:(bk + 1) * 4, :, :],
                                    ps[ii][bk][:],
                                    bias[:, 1, oh:oh + 1], 0.0,
                                    mybir.AluOpType.add, mybir.AluOpType.max)
                        else:
                            # psum = WS^2*conv; out = psum/WS^2 + b  (bf16)
                            if bk == 0:
                                nc.scalar.activation(
                                    dest[oh][:, :, bk * 32:(bk + 1) * 32],
                                    ps[ii][bk][:].rearrange("c pr pc b -> c b (pr pc)"),
                                    AF.Identity,
                                    bias=bias[:, 0, oh:oh + 1], scale=1.0 / WS**2)
                            else:
                                nc.vector.tensor_scalar(
                                    dest[oh][:, :, bk * 32:(bk + 1) * 32],
                                    ps[ii][bk][:].rearrange("c pr pc b -> c b (pr pc)"),
                                    1.0 / WS**2, bias[:, 0, oh:oh + 1],
                                    mybir.AluOpType.mult, mybir.AluOpType.add)
                if wave_cb is not None:
                    wave_cb(oh)

        kh, qh, vh = hh["k"], hh["q"], hh["v"]
        # E_t: [128=(par,64k), oh, hp, b2, 64q] bf16
        E_t = pp.tile([128, 2, 4, BL // 2, NPIX], BF16, name="E_t")
        # VT: [128=(par,64k), b2, h, 33] bf16, col 32 = ones
        VT = pp.tile([128, BL // 2, H, 33], BF16, name="VT")
        nc.vector.memset(VT[:, :, :, 32:33], 1.0)
        # OA: [128=(par,64q), b2, oh, 128c] bf16 (normalized attn out)
        OA = pp.tile([128, BL // 2, 2, 128], BF16, name="OA")
        rcp = pp.tile([128, BL // 2, H], F32, name="rcp")
        concat = [pp.tile([128, BL, NPIX], BF16, name=f"concat{h}")
                  for h in range(2)]
        out_sb = pp.tile([128, 8, 256], F32, name="out_sb")

        with tc.tile_pool(name="pscv", bufs=1, space="PSUM") as cvp:
            # P1: q conv1;  P2: k conv1;  P3: k/q conv2 (wk);
            conv_pass(cvp, w8["q"], [x8["q"]], [("relu", x1["q"], bias_t["q"])], "p1")
            conv_pass(cvp, w8["k"], [x8["k"]], [("relu", x1["k"], bias_t["k"])], "p2")
            conv_pass(cvp, w8["k"], [x1["k"], x1["q"]],
                      [("final", hh["k"], bias_t["k"]),
                       ("final", hh["q"], bias_t["k"])], "p3")

            with tc.tile_pool(name="psqk", bufs=1, space="PSUM") as qkp:
                # ---- QK + exp (overlaps P4/P5 convs) ----
                for oh in range(2):
                    for hp in range(4):
                        pst = qkp.tile([128, BL // 2, 64], F32, tag="pst",
                                       name=f"pst{oh}{hp}", bufs=4)
                        for b2 in range(BL // 2):
                            for par in range(2):
                                b = 2 * b2 + par
                                nc.tensor.matmul(
                                    pst[64 * par:64 * par + 64, b2, :],
                                    kh[oh][hp * 32:(hp + 1) * 32, b, :],
                                    qh[oh][hp * 32:(hp + 1) * 32, b, :],
                                    start=True, stop=True,
                                    tile_position=(32 * hp, 64 * par))
                        nc.scalar.activation(E_t[:, oh, hp, :, :], pst[:],
                                             AF.Exp, scale=RS)

                # P4: v conv1;  P5: v conv2
                conv_pass(cvp, w8["v"], [x8["v"]], [("relu", x1["v"], bias_t["v"])], "p4")
                conv_pass(cvp, w8["v"], [x1["v"]], [("final", hh["v"], bias_t["v"])], "p5")

        # ---- attention tail ----
        with tc.tile_pool(name="pstail", bufs=1, space="PSUM") as psp:
            # vh transposes: per (b2, oh): [128, 2b x 64pix] -> [(par,pix), o]
            for b2 in range(BL // 2):
                for oh in range(2):
                    pvt = psp.tile([128, 128], BF16, tag="ptr",
                                   name=f"pvt{b2}{oh}", bufs=2)
                    nc.tensor.transpose(
                        pvt[:], vh[oh][:, 2 * b2:2 * b2 + 2, :], ident_t[:])
                    nc.vector.tensor_copy(
                        VT[:, b2, oh * 4:(oh + 1) * 4, 0:32],
                        pvt[:].rearrange("k (h d) -> k h d", h=4))

            # AV: per (b2, par, h): E.T @ [vh | ones], 8 head slots per tile
            for b2 in range(BL // 2):
                pso = psp.tile([128, H, 33], F32, tag="pso",
                               name=f"pso{b2}", bufs=4)
                for oh in range(2):
                    for hp in range(4):
                        for par in range(2):
                            nc.tensor.matmul(
                                pso[64 * par:64 * par + 64, oh * 4 + hp, :],
                                E_t[64 * par:64 * par + 64, oh, hp, b2, :],
                                VT[64 * par:64 * par + 64, b2, oh * 4 + hp, :],
                                start=True, stop=True)
                nc.vector.reciprocal(
                    rcp[:, b2, :],
                    pso[:, :, 32:33].rearrange("q h one -> q (h one)"))
                for h in range(H):
                    oh, hp = h // 4, h % 4
                    dst = OA[:, b2, oh, hp * 32:(hp + 1) * 32]
                    src = pso[:, h, 0:32]
                    if h % 2 == 0:
                        nc.scalar.activation(dst, src, AF.Copy,
                                             scale=rcp[:, b2, h:h + 1])
                    else:
                        nc.vector.tensor_scalar_mul(dst, src,
                                                    rcp[:, b2, h:h + 1])

            # attn-out transposes: [(par,q), c] -> [c, (par,q)] -> concat
            for b2 in range(BL // 2):
                for oh in range(2):
                    pot = psp.tile([128, 128], BF16, tag="ptr",
                                   name=f"pot{b2}{oh}", bufs=2)
                    nc.tensor.transpose(pot[:], OA[:, b2, oh, :], ident_t[:])
                    nc.vector.tensor_copy(
                        concat[oh][:, 2 * b2:2 * b2 + 2, :],
                        pot[:].rearrange("c (b q) -> c b q", b=2))

            # output projection: per 128-col block of (b, pix)
            for blk in range(8):
                pspr = psp.tile([128, 256], F32, tag="pspr",
                                name=f"pspr{blk}", bufs=2)
                for oh in range(2):
                    cs = concat[oh][:].rearrange("c b p -> c (b p)")
                    nc.tensor.matmul(
                        pspr[:], cs[:, blk * 128:(blk + 1) * 128],
                        wo_tt[:, oh, :], start=(oh == 0), stop=False)
                nc.tensor.matmul(pspr[:], ones_t[:], bo_t[:],
                                 start=False, stop=True)
                if blk % 2 == 0:
                    nc.vector.tensor_copy(out_sb[:, blk, :], pspr[:])
                else:
                    nc.scalar.copy(out_sb[:, blk, :], pspr[:])
                nc.sync.dma_start(dt["out"][blk], out_sb[:, blk, :])
    nc.compile()
    return nc


# ---------------------------------------------------------------------------
# Host-side prep
# ---------------------------------------------------------------------------
def _to_fp8(a):
    return np.clip(np.asarray(a, np.float32), -240.0, 240.0).astype(
        ml_dtypes.float8_e4m3)


def prep_static(wk, bk, wq, bq, wv, bv, wo, bo):
    st = {}
    for nm, w, b in (("q", wq, bq), ("k", wk, bk), ("v", wv, bv)):
        w = np.asarray(w, np.float32)
        # [128c_lo-part, tap, 2(c-half), 256o] fp8, scaled x256
        wt = np.empty((128, NTAP, 2, 256), ml_dtypes.float8_e4m3)
        for ti, (sr, sc, *_r) in enumerate(TAPS):
            wtap = w[:, :, sr + 7, sc + 7].T * WS     # [c, o]
            wt[:, ti] = _to_fp8(wtap.reshape(2, 128, 256).transpose(1, 0, 2))
        st[f"w{nm}"] = np.ascontiguousarray(wt)
        bb = np.asarray(b, np.float32).reshape(2, 128).T  # [128c, 2oh]
        st[f"bias{nm}"] = np.ascontiguousarray(
            np.stack([bb, WS * bb], axis=1))  # [128, 2kind, 2oh]
    st["wo_t"] = np.ascontiguousarray(
        np.asarray(wo, np.float32).T.reshape(2, 128, 256).transpose(1, 0, 2)
        .astype(ml_dtypes.bfloat16))
    st["bo"] = np.asarray(bo, np.float32).reshape(1, 256)
    st["ones"] = np.ones((1, 128), np.float32)
    st["ident"] = np.eye(128, dtype=ml_dtypes.bfloat16)
    return st


def prep_core_x(x, core):
    """x: [B, 8, 8, D] -> [128, 2, 8, 8, BL] fp8 (c_lo, c_half, pr, pc, b)."""
    xs = np.asarray(x[core * BL:(core + 1) * BL], np.float32)
    xs = xs.transpose(3, 1, 2, 0).reshape(2, 128, 8, 8, BL).transpose(
        1, 0, 2, 3, 4)
    return np.ascontiguousarray(_to_fp8(xs))


def make_in_maps(q, k, v, st):
    in_maps = []
    for core in range(NCORES):
        m = dict(st)
        m["xq"] = prep_core_x(q, core)
        m["xk"] = prep_core_x(k, core)
        m["xv"] = prep_core_x(v, core)
        in_maps.append(m)
    return in_maps


def gather_out(results):
    outs = [r["out"].reshape(BL, 8, 8, D) for r in results]
    return np.concatenate(outs, axis=0)


_NC_CACHE = None


def _get_nc():
    global _NC_CACHE
    if _NC_CACHE is None:
        _NC_CACHE = build_kernel()
    return _NC_CACHE


def kernel(q, k, v, wk, bk, wq, bq, wv, bv, wo, bo):
    nc = _get_nc()
    st = prep_static(wk, bk, wq, bq, wv, bv, wo, bo)
    in_maps = make_in_maps(np.asarray(q), np.asarray(k), np.asarray(v), st)
    res = bass_utils.run_bass_kernel_spmd(
        nc, in_maps, core_ids=list(range(NCORES)))
    return gather_out(res.results)
